# revision 1
# baseline (speedup 1.0000x reference)
"""Trainium2 Bass kernel for GQA attention prefill (nn_Attention_60593398612481).

Full-input contract: kernel(**inputs) takes the unsharded inputs and returns
the full [B, S, DIM] fp32 output. Internally: tensor-parallel across heads on
8 NeuronCores (q-heads 4c..4c+3 + kv-head c on core c; wo row-sharded), each
core computes a full-shape partial of the output projection, host sums the 8
partials (row-parallel "all-reduce" realized at gather time).

Assumes the mask input is the standard causal mask (0 on/below diagonal,
-1e9 above) as produced by the reference setup_inputs().

Layout tricks:
- x is fed pre-transposed (xT), weights column-sharded, so qT/kT/vT come out
  of the QKV matmul with head-dim on partitions — exactly the layout the
  scoresT matmul (k rows on partitions, q free) and the output projection
  (ctxT stationary) need. No on-chip transposes except 128x128 v tiles.
- q/k weight columns are pre-permuted even|odd so RoPE is two half-partition
  DVE muls + one add, fused into the PSUM eviction.
- Scores can't overflow exp (|s| <~ 10), so softmax runs without the
  max-subtraction pass; denominator = ones-matmul partition sum.
- Causal masking: strictly-upper 128x512 blocks are skipped; the diagonal
  128-block needs one triangle-mask multiply; left-masked regions are zeroed
  and excluded from the ctx matmul.
- Attention for batch 0 is emitted mid-phase-1 (with its own small PSUM
  pools) so its exp/mask chains hide under the projection matmuls; batch 1's
  attention overlaps the batch-0 output projection.
"""

import math
from dataclasses import dataclass

import numpy as np
import ml_dtypes

import concourse.bass as bass
import concourse.mybir as mybir
import concourse.tile as tile
from concourse import bacc
from concourse.masks import make_identity

BF16 = mybir.dt.bfloat16
F32 = mybir.dt.float32
AF = mybir.ActivationFunctionType


@dataclass(frozen=True)
class Cfg:
    B: int = 2
    S: int = 2048
    DIM: int = 4096
    NQ: int = 4        # q heads per core
    HD: int = 128
    CB: int = 512      # phase-1 column block (rows of x)
    QBLK: int = 512    # phase-2 q block (PSUM bank)
    KBLK: int = 128    # phase-2 k block (partition dim)
    KBATCH: int = 2    # k blocks per exp batch
    NBLK: int = 512    # phase-3 out-dim block

    @property
    def R(self):
        return self.B * self.S

    @property
    def KT(self):
        return self.DIM // 128

    @property
    def NM(self):
        return self.NQ + 2  # q heads + k + v


def build_nc(cfg: Cfg, reps: int = 1):
    """Build the single-core Bass program (SPMD: same program on 8 cores).

    reps>1 emits the whole computation multiple times (same output buffer) —
    used only for dispatch-overhead-cancelling device-time measurement."""
    nc = bacc.Bacc("TRN2", target_bir_lowering=False)
    B, S, DIM, NQ = cfg.B, cfg.S, cfg.DIM, cfg.NQ
    R, KT, NM = cfg.R, cfg.KT, cfg.NM
    CB, QBLK, KBLK, KBATCH = cfg.CB, cfg.QBLK, cfg.KBLK, cfg.KBATCH
    NBLK = cfg.NBLK
    NCB = R // CB
    ST = S // 128          # seq row-tiles per batch
    DIAG = QBLK // KBLK    # diagonal k-blocks per q-block
    NN = DIM // NBLK

    xT = nc.dram_tensor("xT", [DIM, R], BF16, kind="ExternalInput")
    wqkv = nc.dram_tensor("wqkv", [DIM, NM * 128], BF16, kind="ExternalInput")
    wo = nc.dram_tensor("wo", [NQ * 128, DIM], BF16, kind="ExternalInput")
    cc = nc.dram_tensor("cc", [128, R], BF16, kind="ExternalInput")
    ss = nc.dram_tensor("ss", [128, R], BF16, kind="ExternalInput")
    bm = nc.dram_tensor("bm", [128, 128], BF16, kind="ExternalInput")
    out = nc.dram_tensor("out", [R, DIM], BF16, kind="ExternalOutput")

    with tile.TileContext(nc) as tc:
      for _rep in range(reps):
        with (
            tc.tile_pool(name="const", bufs=1) as constp,
            tc.tile_pool(name="qkv", bufs=1) as qkvp,
            tc.tile_pool(name="ctx", bufs=1) as ctxp,
            tc.tile_pool(name="expp", bufs=3) as expp,
            tc.tile_pool(name="dnp", bufs=2) as dnp,
            tc.tile_pool(name="nrm", bufs=2) as nrmp,
        ):
            # ---- constants (DMAs issued inside phase 1, after w/x) ----
            bm_sb = constp.tile([128, 128], BF16)
            ident = constp.tile([128, 128], BF16)
            ones_sb = constp.tile([128, 1], BF16)
            make_identity(nc, ident)
            nc.vector.memset(ones_sb[:], 1.0)

            # ---- persistent activations ----
            qkT = qkvp.tile([128, NQ + 1, R], BF16)   # roped qT (4 heads) + kT
            v_sb = qkvp.tile([128, R // 128, 128], BF16)  # v natural, row tiles
            ctxT = ctxp.tile([128, NQ, R], BF16)

            def emit_attention(b, scp, cxp):
                """ScoresT-orientation flash attention for batch b."""
                for h in range(NQ):
                    qh = qkT[:, h, b * S:(b + 1) * S]
                    kh = qkT[:, NQ, b * S:(b + 1) * S]
                    for j in range(S // QBLK):
                        cx = cxp.tile([128, QBLK], F32, tag="cx")
                        dn = dnp.tile([128, QBLK], BF16, tag="dn")
                        nc.vector.memset(dn[:], 0.0)
                        nkb = (j + 1) * QBLK // KBLK
                        for kb0 in range(0, nkb, KBATCH):
                            nb = min(KBATCH, nkb - kb0)
                            sc = scp.tile([128, KBATCH, QBLK], F32, tag="sc")
                            for i in range(nb):
                                kb = kb0 + i
                                nc.tensor.matmul(
                                    sc[:, i, :],
                                    kh[:, kb * KBLK:(kb + 1) * KBLK],
                                    qh[:, j * QBLK:(j + 1) * QBLK],
                                    start=True, stop=True,
                                )
                            ex = expp.tile([128, KBATCH, QBLK], BF16, tag="ex")
                            nc.scalar.activation(
                                ex[:, 0:nb, :], sc[:, 0:nb, :], AF.Exp
                            )
                            for i in range(nb):
                                rel = (kb0 + i) - j * DIAG
                                if 0 <= rel < DIAG:
                                    if rel > 0:
                                        nc.vector.memset(
                                            ex[:, i, 0:rel * KBLK], 0.0
                                        )
                                    nc.vector.tensor_mul(
                                        ex[:, i, rel * KBLK:(rel + 1) * KBLK],
                                        ex[:, i, rel * KBLK:(rel + 1) * KBLK],
                                        bm_sb[:],
                                    )
                            for i in range(nb):
                                kb = kb0 + i
                                rel = kb - j * DIAG
                                c0 = rel * KBLK if 0 < rel < DIAG else 0
                                nc.vector.tensor_add(
                                    dn[:, c0:], dn[:, c0:], ex[:, i, c0:]
                                )
                                nc.tensor.matmul(
                                    cx[:, c0:],
                                    v_sb[:, b * ST + kb, :],
                                    ex[:, i, c0:],
                                    start=(kb == 0), stop=(kb == nkb - 1),
                                )
                        # softmax denominator: ones-matmul partition sum
                        # (psum slot borrowed from the scores pool)
                        dsp = scp.tile([1, QBLK], F32, tag="sc")
                        nc.tensor.matmul(
                            dsp[:], ones_sb[:], dn[:], start=True, stop=True
                        )
                        rec = nrmp.tile([1, QBLK], F32, tag="rec")
                        recb = nrmp.tile([128, QBLK], F32, tag="recb")
                        nc.vector.reciprocal(rec[:], dsp[:])
                        nc.gpsimd.partition_broadcast(recb[:], rec[:])
                        nc.vector.tensor_mul(
                            ctxT[:, h, b * S + j * QBLK:b * S + (j + 1) * QBLK],
                            cx[:], recb[:],
                        )

            # ===== Phase 1 (QKV projection) =====
            # PSUM: p1 6 + vtranspose 2 = 8 banks.
            with (
                tc.tile_pool(name="wq", bufs=1) as wp,
                tc.tile_pool(name="xin", bufs=3) as xp,
                tc.tile_pool(name="p1ps", bufs=6, space="PSUM") as p1ps,
                tc.tile_pool(name="tps", bufs=2, space="PSUM") as tps,
                tc.tile_pool(name="p1tmp", bufs=2) as p1tmp,
                tc.tile_pool(name="csp", bufs=2) as csp,
                tc.tile_pool(name="vtp", bufs=2) as vtp,
            ):
                w_sb = wp.tile([128, NM, KT, 128], BF16)
                wqkv_r = wqkv.rearrange("(kt p) (m j) -> p m kt j", p=128, j=128)
                xT_r = xT.rearrange("(kt p) r -> p kt r", p=128)
                KTH = KT // 2
                for cb in range(NCB):
                    csl = slice(cb * CB, (cb + 1) * CB)
                    xcb0 = xp.tile([128, KTH, CB], BF16, tag="xcb")
                    xcb1 = xp.tile([128, KTH, CB], BF16, tag="xcb")
                    nc.sync.dma_start(out=xcb0[:], in_=xT_r[:, 0:KTH, csl])
                    if cb == 0:
                        # weights interleaved so m=0 lands right after xcb0
                        nc.sync.dma_start(out=w_sb[:, 0], in_=wqkv_r[:, 0])
                        nc.sync.dma_start(out=xcb1[:], in_=xT_r[:, KTH:KT, csl])
                        for m in range(1, NM):
                            nc.sync.dma_start(out=w_sb[:, m], in_=wqkv_r[:, m])
                        nc.sync.dma_start(out=bm_sb[:], in_=bm[:])
                    else:
                        nc.sync.dma_start(out=xcb1[:], in_=xT_r[:, KTH:KT, csl])
                    # cos/sin streamed per-cb, just in time for rope eviction
                    cct = csp.tile([128, CB], BF16, tag="cc")
                    sst = csp.tile([128, CB], BF16, tag="ss")
                    nc.sync.dma_start(out=cct[:], in_=cc[:, csl])
                    nc.sync.dma_start(out=sst[:], in_=ss[:, csl])
                    vstage = vtp.tile([128, CB], BF16, tag="vt")
                    for m in range(NM):
                        ps = p1ps.tile([128, CB], F32, tag="p1")
                        for kt in range(KT):
                            xsrc = xcb0 if kt < KTH else xcb1
                            nc.tensor.matmul(
                                ps[:], w_sb[:, m, kt, :], xsrc[:, kt % KTH, :],
                                start=(kt == 0), stop=(kt == KT - 1),
                            )
                        if m < NQ + 1:
                            # RoPE fused into eviction (even|odd permuted):
                            # out = ps*cc + swap_halves(ps)*ss
                            t2 = p1tmp.tile([128, CB], BF16, tag="t2")
                            nc.vector.tensor_mul(
                                t2[0:64, :], ps[64:128, :], sst[0:64, :]
                            )
                            nc.vector.tensor_mul(
                                t2[64:128, :], ps[0:64, :], sst[64:128, :]
                            )
                            dst = qkT[:, m, csl]
                            nc.vector.tensor_mul(dst, ps[:], cct[:])
                            nc.vector.tensor_add(dst, dst, t2[:])
                        else:
                            nc.vector.tensor_copy(vstage[:], ps[:])
                    # transpose this block's v tiles to natural layout
                    for ti in range(CB // 128):
                        t = cb * (CB // 128) + ti
                        pt = tps.tile([128, 128], BF16, tag="tp")
                        nc.tensor.transpose(
                            pt[:], vstage[:, ti * 128:(ti + 1) * 128], ident[:]
                        )
                        nc.any.tensor_copy(v_sb[:, t, :], pt[:])

            # ===== wo load + attention batch 1 + output projection =====
            # PSUM: sc_b 4 + cx_b 2 + p3 2 = 8 banks.
            with tc.tile_pool(name="wo", bufs=1) as wop:
                wo_sb = wop.tile([128, NQ, DIM], BF16)
                nc.sync.dma_start(
                    out=wo_sb[:], in_=wo.rearrange("(h p) n -> p h n", p=128)
                )
                with (
                    tc.tile_pool(name="scpsb", bufs=2, space="PSUM") as scps_b,
                    tc.tile_pool(name="cxpsb", bufs=2, space="PSUM") as cxps_b,
                    tc.tile_pool(name="p3ps", bufs=2, space="PSUM") as p3ps,
                    tc.tile_pool(name="p3o", bufs=3) as p3o,
                ):
                    for b in range(B):
                        emit_attention(b, scps_b, cxps_b)
                    # phase 3: fills PE stalls of batch-1 attention; the
                    # second half alternates into the cx pool (idle by then)
                    gidx = 0
                    for r in range(R // 128):
                        for n in range(NN):
                            if r >= (R // 256) and gidx % 2 == 1:
                                ps = cxps_b.tile([128, NBLK], F32, tag="cx")
                            else:
                                ps = p3ps.tile([128, NBLK], F32, tag="p3")
                            gidx += 1
                            for h in range(NQ):
                                nc.tensor.matmul(
                                    ps[:],
                                    ctxT[:, h, r * 128:(r + 1) * 128],
                                    wo_sb[:, h, n * NBLK:(n + 1) * NBLK],
                                    start=(h == 0), stop=(h == NQ - 1),
                                )
                            ob = p3o.tile([128, NBLK], BF16, tag="ob")
                            nc.any.tensor_copy(ob[:], ps[:])
                            nc.sync.dma_start(
                                out=out[r * 128:(r + 1) * 128,
                                        n * NBLK:(n + 1) * NBLK],
                                in_=ob[:],
                            )
    nc.compile()
    return nc


# ---------------- host-side sharding ----------------

_EO_PERM = np.concatenate([np.arange(0, 128, 2), np.arange(1, 128, 2)])


def shard_inputs(cfg: Cfg, x, wq, wk, wv, wo, freqs_cos, freqs_sin, mask,
                 n_cores: int):
    """Build per-core input maps (numpy, bf16)."""
    bf = ml_dtypes.bfloat16
    B, S, DIM, NQ, HD = cfg.B, cfg.S, cfg.DIM, cfg.NQ, cfg.HD
    R = cfg.R
    x2 = np.asarray(x, np.float32).reshape(R, DIM)
    xT = np.ascontiguousarray(x2.T).astype(bf)

    scale = 1.0 / math.sqrt(HD)
    wq = np.asarray(wq, np.float32) * scale
    wk = np.asarray(wk, np.float32)
    wv = np.asarray(wv, np.float32)
    wo = np.asarray(wo, np.float32)

    cosT = np.asarray(freqs_cos, np.float32).T  # [64, S]
    sinT = np.asarray(freqs_sin, np.float32).T
    cc1 = np.concatenate([cosT, cosT], axis=0)          # [128, S]
    ss1 = np.concatenate([-sinT, sinT], axis=0)
    cc = np.tile(cc1, (1, B)).astype(bf)                # [128, R]
    ss = np.tile(ss1, (1, B)).astype(bf)

    m = np.asarray(mask, np.float32)
    bm = (m[:128, :128].T == 0.0).astype(bf)            # allowed -> 1

    in_maps = []
    for c in range(n_cores):
        qcols = []
        for i in range(NQ):
            h = c * NQ + i
            qcols.append(wq[:, h * HD:(h + 1) * HD][:, _EO_PERM])
        kcol = wk[:, c * HD:(c + 1) * HD][:, _EO_PERM]
        vcol = wv[:, c * HD:(c + 1) * HD]
        wqkv = np.concatenate(qcols + [kcol, vcol], axis=1).astype(bf)
        wo_c = wo[c * NQ * HD:(c + 1) * NQ * HD, :].astype(bf)
        in_maps.append({
            "xT": xT, "wqkv": wqkv, "wo": wo_c,
            "cc": cc, "ss": ss, "bm": bm,
        })
    return in_maps


_NC_CACHE = {}


def _get_nc(cfg: Cfg):
    if cfg not in _NC_CACHE:
        _NC_CACHE[cfg] = build_nc(cfg)
    return _NC_CACHE[cfg]


def kernel(x, wq, wk, wv, wo, freqs_cos, freqs_sin, mask, start_pos=0,
           **_ignored):
    from concourse.bass_utils import run_bass_kernel_spmd

    cfg = Cfg()
    nc = _get_nc(cfg)
    in_maps = shard_inputs(cfg, x, wq, wk, wv, wo, freqs_cos, freqs_sin, mask,
                           n_cores=8)
    res = run_bass_kernel_spmd(nc, in_maps, core_ids=list(range(8)))
    acc = np.zeros((cfg.R, cfg.DIM), np.float32)
    for c in range(8):
        acc += res.results[c]["out"].astype(np.float32)
    return acc.reshape(cfg.B, cfg.S, cfg.DIM)

